# revision 28
# baseline (speedup 1.0000x reference)
"""Mask R-CNN paste_masks_in_image kernel for Trainium2 (8 NeuronCores).

out[n] = Y_n.T @ mask_n @ X_n  (separable bilinear paste), computed and
written only on each box's window:

 - Host computes interp matrices, finds each instance's nonzero row/col
   window, sorts each core's 16 instances by row-span and assigns them to
   16 static SLOTS.  Slot k's geometry (row budget RHT_k, col width L=304)
   is the max over the 8 cores, so ONE SPMD program serves all cores.
 - Stage 1 (PE, bf16): one block-diagonal matmul per group of 4 slots
   computes mxT = mask.T @ Ywin for all 4 at once ([112, WG] PSUM).
 - Stage 2 (PE, bf16): per 128-row tile, window = mxT_tile.T @ Xwin
   ([128, L] f32 PSUM), copied into a packed SBUF payload.
 - Output: per slot, ONE dma_start with a RUNTIME destination offset
   (value_load from an input tensor) writes the [RHT_k, L] window into
   the full [ni, img_h, img_w] output.  Rows/cols beyond the true span
   carry zeros (zero-padded interp weights) and land inside the image,
   so they are harmless.  The runner pre-zeros/donates output buffers,
   so unwritten pixels stay zero.
 - Falls back to a dense full-image writer for out-of-distribution
   inputs (tiny images or spans wider than the static budgets).
"""
import sys

if "/opt/trn_rl_repo" not in sys.path:
    sys.path.insert(0, "/opt/trn_rl_repo")

import numpy as np

N_CORES = 8
HM = WM = 28
L = 304           # static column window (covers max cw ~298)
TILE_H = 128      # rows per payload tile / matmul partition budget
GSLOT = 2         # slots per stage-1 block-diagonal matmul
PB = 32           # partition stride between slot blocks (matmul base rule)
KD = PB * (GSLOT - 1) + HM   # 60 partitions for the paired stage-1

_BUILD_CACHE = {}
_ws_ctr = [0]


def _split_multi_waits(nc):
    """This image's walrus allows only ONE sync-wait per instruction; hoist
    extra waits onto preceding NoOps on the same engine."""
    import concourse.mybir as mybir

    for fn in nc.m.functions:
        for blk in fn.blocks:
            insts = list(blk.instructions)
            out = []
            changed = False
            for inst in insts:
                si = getattr(inst, "sync_info", None)
                waits = list(si.on_wait) if (si is not None and si.on_wait) else []
                if len(waits) > 1:
                    changed = True
                    for w in waits[:-1]:
                        _ws_ctr[0] += 1
                        out.append(
                            mybir.InstNoOp(
                                name=f"waitsplit-{_ws_ctr[0]}",
                                engine=inst.engine,
                                sync_info=mybir.SyncInfo(on_wait=[w], on_update=[]),
                            )
                        )
                    si.on_wait = [waits[-1]]
                out.append(inst)
            if changed:
                try:
                    blk.instructions = out
                except Exception:
                    del blk.instructions[:]
                    blk.instructions.extend(out)


def _interp_mats(p0, p1, out_size, mask_size):
    """W[n, k, j] = w0*(i0==k) + w1*(i0+1==k); exact f32 replication of the
    reference's align_corners=False bilinear weights with zero padding."""
    xs = (np.arange(out_size, dtype=np.float32) + np.float32(0.5))[None, :]
    g = (xs - p0[:, None]) / (p1 - p0)[:, None] * np.float32(2) - np.float32(1)
    p = (g + np.float32(1)) * np.float32(mask_size * 0.5) - np.float32(0.5)
    f = np.floor(p)
    i0 = f.astype(np.int64)
    w1 = (p - f).astype(np.float32)
    w0 = np.float32(1.0) - w1
    ks = np.arange(mask_size, dtype=np.int64)[None, :, None]
    W = (i0[:, None, :] == ks) * w0[:, None, :] + ((i0 + 1)[:, None, :] == ks) * w1[
        :, None, :
    ]
    return np.ascontiguousarray(W.astype(np.float32))


def _scaled_boxes(boxes, img_h, img_w, in_h, in_w):
    sx = np.float32(img_w / in_w)
    sy = np.float32(img_h / in_h)
    b = boxes.astype(np.float32) * np.array([sx, sy, sx, sy], np.float32)
    x0 = np.clip(b[:, 0], np.float32(0.0), np.float32(img_w))
    y0 = np.clip(b[:, 1], np.float32(0.0), np.float32(img_h))
    x1 = np.clip(b[:, 2], np.float32(0.0), np.float32(img_w))
    y1 = np.clip(b[:, 3], np.float32(0.0), np.float32(img_h))
    return x0, y0, x1, y1


def _prep_common(masks, boxes, img_h, img_w, in_h, in_w):
    x0, y0, x1, y1 = _scaled_boxes(boxes, img_h, img_w, in_h, in_w)
    xmat = _interp_mats(x0, x1, img_w, WM)   # [N, 28, img_w]
    ytmat = _interp_mats(y0, y1, img_h, HM)  # [N, 28, img_h]
    return xmat, ytmat


def _spans(mat):
    """Per-instance [start, span] of nonzero columns of mat [N, 28, S]."""
    n = mat.shape[0]
    s0 = np.zeros(n, np.int64)
    sl = np.zeros(n, np.int64)
    nz = mat.any(axis=1)
    for i in range(n):
        w = np.flatnonzero(nz[i])
        if w.size:
            s0[i], sl[i] = w[0], w[-1] - w[0] + 1
    return s0, sl


def _slot_geometry(rh_by_core):
    """rh_by_core [M, ni] row-spans -> per-core slot order + static slot
    row budgets RHT (multiple of TILE_H).  Slot pairs are arranged
    mid-size first, largest next, smallest last, so output DMAs start
    flowing early and the final transfer is small."""
    order = np.argsort(-rh_by_core, axis=1, kind="stable")   # [M, ni]
    ni = rh_by_core.shape[1]
    npair = ni // GSLOT
    if npair >= 4:
        pperm = [npair - 1, 0, 1] + list(range(2, npair - 1))
    else:
        pperm = list(range(npair))
    sperm = np.concatenate(
        [np.arange(p * GSLOT, (p + 1) * GSLOT) for p in pperm]
    )
    order = order[:, sperm]
    sorted_rh = np.take_along_axis(rh_by_core, order, axis=1)
    rh_slot = sorted_rh.max(axis=0)                          # [ni]
    tiles = np.maximum((rh_slot + TILE_H - 1) // TILE_H, 1)
    rht = tiles * TILE_H
    return order, rht.astype(int), tiles.astype(int)


def _build_slotted(ni, img_h, img_w, rht, tiles):
    import concourse.bass as bass
    import concourse.mybir as mybir
    from concourse.tile import TileContext

    f32 = mybir.dt.float32
    bf16 = mybir.dt.bfloat16
    i32 = mybir.dt.int32
    nslot = len(rht)
    ngrp = nslot // GSLOT
    wg = [int(max(rht[g * GSLOT : (g + 1) * GSLOT])) for g in range(ngrp)]
    # my (=mblk|ybuf) column layout per group: [mask blockdiag KD | Y, KD x WG]
    myoff = []
    o = 0
    for g in range(ngrp):
        myoff.append(o)
        o += KD + wg[g]
    mytot = o
    segoff = np.concatenate([[0], np.cumsum(tiles)]).astype(int)
    nseg = int(segoff[-1])

    nc = bass.Bass()
    my_d = nc.dram_tensor("my", [KD, mytot], bf16, kind="ExternalInput")
    x_d = nc.dram_tensor("xb", [KD, ngrp * L], bf16, kind="ExternalInput")
    off_d = nc.dram_tensor("offs", [1, nslot], i32, kind="ExternalInput")
    out_d = nc.dram_tensor("out", [ni, img_h, img_w], f32, kind="ExternalOutput")

    with TileContext(nc) as tc:
        with (
            tc.tile_pool(name="my", bufs=1) as myp,
            tc.tile_pool(name="xb", bufs=1) as xbp,
            tc.tile_pool(name="of", bufs=1) as ofp,
            tc.tile_pool(name="mx", bufs=1) as mxp,
            tc.tile_pool(name="pay", bufs=1) as payp,
            tc.tile_pool(name="psA", bufs=2, space="PSUM") as psa,
            tc.tile_pool(name="psB", bufs=3, space="PSUM") as psb,
        ):
            myt = myp.tile([KD, mytot], bf16, tag="my")
            xbt = xbp.tile([KD, ngrp * L], bf16, tag="xb")
            oft = ofp.tile([1, nslot], i32, tag="of")
            mxT = mxp.tile([KD, int(sum(wg))], bf16, tag="mxT")
            pay = payp.tile([TILE_H, nseg * L], f32, tag="pay")

            # my (stage-1 inputs) on sync, 2 groups per slice, in processing
            # order; xb + offs on scalar: the first matmul only waits for
            # its own my slice.
            half = (ngrp // 2) * L
            for h in range(0, ngrp, 2):
                a = myoff[h]
                b = myoff[h + 1] + KD + wg[h + 1]
                nc.sync.dma_start(out=myt[:, a:b], in_=my_d[:, a:b])
            nc.scalar.dma_start(out=xbt[:, :half], in_=x_d[:, :half])
            nc.scalar.dma_start(out=xbt[:, half:], in_=x_d[:, half:])
            nc.scalar.dma_start(out=oft[:], in_=off_d[:])

            # Prefetch all destination offsets into engine registers up
            # front (value_load is ~0.7us) so out-DMA issue is just the
            # dma_start.  The register's engine must issue that slot's DMA.
            dengs = [(nc.sync, nc.gpsimd, nc.scalar, nc.gpsimd)[k % 4] for k in range(nslot)]
            rvs = [dengs[k].value_load(oft[0:1, k : k + 1]) for k in range(nslot)]

            import bass_rust

            cpi = 0
            mxo = [0]
            for g in range(1, ngrp):
                mxo.append(mxo[-1] + wg[g - 1])
            for g in range(ngrp):
                a = myoff[g]
                pa = psa.tile([KD, 512], f32, tag="pa")
                nc.tensor.matmul(
                    out=pa[:, : wg[g]],
                    lhsT=myt[:, a : a + KD],
                    rhs=myt[:, a + KD : a + KD + wg[g]],
                    start=True,
                    stop=True,
                )
                enga = nc.scalar.copy if g % 2 == 0 else nc.vector.tensor_copy
                enga(out=mxT[:, mxo[g] : mxo[g] + wg[g]], in_=pa[:, : wg[g]])
                for i in range(GSLOT):
                    k = g * GSLOT + i
                    nt = int(tiles[k])
                    s0 = int(segoff[k])
                    # pairs of row-tiles share one 2-bank PSUM tile and one
                    # strided copy into the payload
                    for t0 in range(0, nt, 2):
                        npair = min(2, nt - t0)
                        pb = psb.tile([TILE_H, 1024], f32, tag="pb")
                        for h in range(npair):
                            t = t0 + h
                            nc.tensor.matmul(
                                out=pb[:, h * 512 : h * 512 + L],
                                lhsT=mxT[
                                    i * PB : i * PB + HM,
                                    mxo[g] + t * TILE_H : mxo[g] + (t + 1) * TILE_H,
                                ],
                                rhs=xbt[i * PB : i * PB + HM, g * L : (g + 1) * L],
                                start=True,
                                stop=True,
                            )
                        s = s0 + t0
                        engb = (nc.vector.tensor_copy, nc.scalar.copy, nc.vector.tensor_copy)[cpi % 3]
                        cpi += 1
                        if npair == 2:
                            src_ps = pb[:, 0:L].copy()
                            src_ps.ap = bass_rust.VecI64Pair(
                                [[1024, TILE_H], [512, 2], [1, L]]
                            )
                            dst = pay[:, s * L : (s + 2) * L].rearrange(
                                "p (b c) -> p b c", b=2
                            )
                            engb(out=dst, in_=src_ps)
                        else:
                            engb(out=pay[:, s * L : (s + 1) * L], in_=pb[:, :L])
                    # one windowed write per slot: [RHT_k, L] at runtime offset
                    tmpl = out_d[0, 0 : nt * TILE_H, 0:L].rearrange(
                        "(t p) l -> p t l", p=TILE_H
                    )
                    dyn = bass.AP(
                        tensor=tmpl.tensor,
                        offset=rvs[k],
                        ap=tmpl.ap,
                        dep_tracking_offset=k * 3 * TILE_H * img_w,
                    )
                    src = pay[:, s0 * L : (s0 + nt) * L].rearrange(
                        "p (t l) -> p t l", l=L
                    )
                    dengs[k].dma_start(out=dyn, in_=src)
    _split_multi_waits(nc)
    return nc


def _build_dense(ni, img_h, img_w):
    """Fallback: writes every output pixel (no window assumption)."""
    import concourse.bass as bass
    import concourse.mybir as mybir
    from concourse.tile import TileContext

    f32 = mybir.dt.float32
    f32r = mybir.dt.float32r
    nc = bass.Bass()
    maskT_d = nc.dram_tensor("maskT", [ni, WM, HM], f32r, kind="ExternalInput")
    x_d = nc.dram_tensor("xmat", [ni, WM, img_w], f32r, kind="ExternalInput")
    yt_d = nc.dram_tensor("ytmat", [ni, HM, img_h], f32r, kind="ExternalInput")
    out_d = nc.dram_tensor("out", [ni, img_h, img_w], f32, kind="ExternalOutput")
    chunks = []
    c = 0
    while c < img_w:
        cw = min(512, img_w - c)
        chunks.append((c, cw))
        c += cw
    rtiles = []
    r = 0
    while r < img_h:
        rh = min(128, img_h - r)
        rtiles.append((r, rh))
        r += rh

    with TileContext(nc) as tc:
        with (
            tc.tile_pool(name="w", bufs=3) as wp,
            tc.tile_pool(name="mx", bufs=2) as mxp,
            tc.tile_pool(name="psA", bufs=2, space="PSUM") as psa,
            tc.tile_pool(name="psB", bufs=2, space="PSUM") as psb,
            tc.tile_pool(name="ob", bufs=4) as obp,
        ):
            for n in range(ni):
                mT = wp.tile([WM, HM], f32r, tag="mT")
                xt = wp.tile([WM, img_w], f32r, tag="xt")
                yt = wp.tile([HM, img_h], f32r, tag="yt")
                nc.sync.dma_start(out=mT[:], in_=maskT_d[n])
                nc.sync.dma_start(out=xt[:], in_=x_d[n])
                nc.sync.dma_start(out=yt[:], in_=yt_d[n])

                mx = mxp.tile([HM, img_w], f32r, tag="mx")
                for j, (c0, cw) in enumerate(chunks):
                    pa = psa.tile([HM, 512], f32, tag="pa")
                    nc.tensor.matmul(
                        out=pa[:, :cw], lhsT=mT[:], rhs=xt[:, c0 : c0 + cw],
                        start=True, stop=True,
                    )
                    if j % 2 == 0:
                        nc.vector.tensor_copy(out=mx[:, c0 : c0 + cw], in_=pa[:, :cw])
                    else:
                        nc.scalar.copy(out=mx[:, c0 : c0 + cw], in_=pa[:, :cw])

                for r0, rh in rtiles:
                    pb = psb.tile([128, 3 * 512], f32, tag="pb")
                    for k, (c0, cw) in enumerate(chunks):
                        nc.tensor.matmul(
                            out=pb[:rh, k * 512 : k * 512 + cw],
                            lhsT=yt[:, r0 : r0 + rh],
                            rhs=mx[:, c0 : c0 + cw],
                            start=True, stop=True,
                        )
                    ob = obp.tile([128, img_w], f32, tag="ob")
                    for k, (c0, cw) in enumerate(chunks):
                        eng = nc.vector.tensor_copy if k % 2 == 0 else nc.scalar.copy
                        eng(out=ob[:rh, c0 : c0 + cw], in_=pb[:rh, k * 512 : k * 512 + cw])
                    nc.sync.dma_start(out=out_d[n, r0 : r0 + rh, :], in_=ob[:rh, :])
    _split_multi_waits(nc)
    return nc


def _run(masks, boxes, img_h, img_w, in_h, in_w, trace=False):
    n = masks.shape[0]
    assert n % N_CORES == 0
    ni = n // N_CORES
    xmat, ytmat = _prep_common(masks, boxes, img_h, img_w, in_h, in_w)
    c0s, cws = _spans(xmat)
    r0s, rhs_ = _spans(ytmat)

    fits = (
        ni % GSLOT == 0
        and img_w >= L
        and cws.max(initial=0) <= L
        and rhs_.max(initial=0) <= 3 * TILE_H
        and img_h >= 3 * TILE_H
    )
    if fits:
        rh_by_core = rhs_.reshape(N_CORES, ni)
        order, rht, tiles = _slot_geometry(rh_by_core)
        if rht.max() <= img_h:
            return _run_slotted(
                masks, xmat, ytmat, c0s, r0s, rhs_, order, rht, tiles,
                ni, img_h, img_w, trace,
            )
    return _run_dense(masks, xmat, ytmat, ni, img_h, img_w, trace)


def _run_slotted(masks, xmat, ytmat, c0s, r0s, rhs_, order, rht, tiles,
                 ni, img_h, img_w, trace):
    import ml_dtypes
    from concourse.bass_utils import run_bass_kernel_spmd

    bf16 = ml_dtypes.bfloat16
    key = ("slot", ni, img_h, img_w, tuple(rht))
    if key not in _BUILD_CACHE:
        _BUILD_CACHE[key] = _build_slotted(ni, img_h, img_w, rht, tiles)
    nc = _BUILD_CACHE[key]

    nslot = ni
    ngrp = nslot // GSLOT
    wg = [int(max(rht[g * GSLOT : (g + 1) * GSLOT])) for g in range(ngrp)]
    myoff = []
    o = 0
    for g in range(ngrp):
        myoff.append(o)
        o += KD + wg[g]
    mytot = o

    in_maps = []
    for c in range(N_CORES):
        my = np.zeros((KD, mytot), bf16)
        xb = np.zeros((KD, ngrp * L), bf16)
        offs = np.zeros((1, nslot), np.int32)
        for k in range(nslot):
            g, i = divmod(k, GSLOT)
            n = c * ni + int(order[c, k])
            nloc = int(order[c, k])
            a = myoff[g]
            p0 = i * PB
            my[p0 : p0 + HM, a + p0 : a + p0 + HM] = masks[n, 0].astype(bf16)
            rr = min(int(r0s[n]), img_h - int(rht[k]))
            take = min(wg[g], img_h - rr)
            my[p0 : p0 + HM, a + KD : a + KD + take] = ytmat[n][
                :, rr : rr + take
            ].astype(bf16)
            cc = min(int(c0s[n]), img_w - L)
            xb[p0 : p0 + HM, g * L : (g + 1) * L] = xmat[n][:, cc : cc + L].astype(
                bf16
            )
            offs[0, k] = nloc * img_h * img_w + rr * img_w + cc
        in_maps.append({"my": my, "xb": xb, "offs": offs})

    res = run_bass_kernel_spmd(nc, in_maps, core_ids=list(range(N_CORES)), trace=trace)
    out = np.concatenate([res.results[c]["out"] for c in range(N_CORES)], axis=0)
    return out, res


def _run_dense(masks, xmat, ytmat, ni, img_h, img_w, trace):
    from concourse.bass_utils import run_bass_kernel_spmd

    key = ("dense", ni, img_h, img_w)
    if key not in _BUILD_CACHE:
        _BUILD_CACHE[key] = _build_dense(ni, img_h, img_w)
    nc = _BUILD_CACHE[key]
    maskt = np.ascontiguousarray(
        np.transpose(masks[:, 0].astype(np.float32), (0, 2, 1))
    )
    in_maps = []
    for c in range(N_CORES):
        s = slice(c * ni, (c + 1) * ni)
        in_maps.append({"maskT": maskt[s], "xmat": xmat[s], "ytmat": ytmat[s]})
    res = run_bass_kernel_spmd(nc, in_maps, core_ids=list(range(N_CORES)), trace=trace)
    out = np.concatenate([res.results[c]["out"] for c in range(N_CORES)], axis=0)
    return out, res


def kernel(masks, boxes, img_h, img_w, in_h, in_w):
    img_h, img_w, in_h, in_w = int(img_h), int(img_w), int(in_h), int(in_w)
    masks = np.asarray(masks, dtype=np.float32)
    boxes = np.asarray(boxes, dtype=np.float32)
    out, _ = _run(masks, boxes, img_h, img_w, in_h, in_w, trace=False)
    return out


# revision 30
# speedup vs baseline: 1.0513x; 1.0513x over previous
"""Mask R-CNN paste_masks_in_image kernel for Trainium2 (8 NeuronCores).

out[n] = Y_n.T @ mask_n @ X_n  (separable bilinear paste), computed and
written only on each box's window:

 - Host computes interp matrices, finds each instance's nonzero row/col
   window, sorts each core's 16 instances by row-span and assigns them to
   16 static SLOTS.  Slot k's geometry (row budget RHT_k, col width L=304)
   is the max over the 8 cores, so ONE SPMD program serves all cores.
 - Stage 1 (PE, bf16): one block-diagonal matmul per group of 4 slots
   computes mxT = mask.T @ Ywin for all 4 at once ([112, WG] PSUM).
 - Stage 2 (PE, bf16): per 128-row tile, window = mxT_tile.T @ Xwin
   ([128, L] f32 PSUM), copied into a packed SBUF payload.
 - Output: per slot, ONE dma_start with a RUNTIME destination offset
   (value_load from an input tensor) writes the [RHT_k, L] window into
   the full [ni, img_h, img_w] output.  Rows/cols beyond the true span
   carry zeros (zero-padded interp weights) and land inside the image,
   so they are harmless.  The runner pre-zeros/donates output buffers,
   so unwritten pixels stay zero.
 - Falls back to a dense full-image writer for out-of-distribution
   inputs (tiny images or spans wider than the static budgets).
"""
import sys

if "/opt/trn_rl_repo" not in sys.path:
    sys.path.insert(0, "/opt/trn_rl_repo")

import numpy as np

N_CORES = 8
HM = WM = 28
L = 304           # static column window (covers max cw ~298)
TILE_H = 128      # rows per payload tile / matmul partition budget
GSLOT = 2         # slots per stage-1 block-diagonal matmul
PB = 32           # partition stride between slot blocks (matmul base rule)
KD = PB * (GSLOT - 1) + HM   # 60 partitions for the paired stage-1

_BUILD_CACHE = {}
_ws_ctr = [0]


def _split_multi_waits(nc):
    """This image's walrus allows only ONE sync-wait per instruction; hoist
    extra waits onto preceding NoOps on the same engine."""
    import concourse.mybir as mybir

    for fn in nc.m.functions:
        for blk in fn.blocks:
            insts = list(blk.instructions)
            out = []
            changed = False
            for inst in insts:
                si = getattr(inst, "sync_info", None)
                waits = list(si.on_wait) if (si is not None and si.on_wait) else []
                if len(waits) > 1:
                    changed = True
                    for w in waits[:-1]:
                        _ws_ctr[0] += 1
                        out.append(
                            mybir.InstNoOp(
                                name=f"waitsplit-{_ws_ctr[0]}",
                                engine=inst.engine,
                                sync_info=mybir.SyncInfo(on_wait=[w], on_update=[]),
                            )
                        )
                    si.on_wait = [waits[-1]]
                out.append(inst)
            if changed:
                try:
                    blk.instructions = out
                except Exception:
                    del blk.instructions[:]
                    blk.instructions.extend(out)


def _interp_mats(p0, p1, out_size, mask_size):
    """W[n, k, j] = w0*(i0==k) + w1*(i0+1==k); exact f32 replication of the
    reference's align_corners=False bilinear weights with zero padding."""
    xs = (np.arange(out_size, dtype=np.float32) + np.float32(0.5))[None, :]
    g = (xs - p0[:, None]) / (p1 - p0)[:, None] * np.float32(2) - np.float32(1)
    p = (g + np.float32(1)) * np.float32(mask_size * 0.5) - np.float32(0.5)
    f = np.floor(p)
    i0 = f.astype(np.int64)
    w1 = (p - f).astype(np.float32)
    w0 = np.float32(1.0) - w1
    ks = np.arange(mask_size, dtype=np.int64)[None, :, None]
    W = (i0[:, None, :] == ks) * w0[:, None, :] + ((i0 + 1)[:, None, :] == ks) * w1[
        :, None, :
    ]
    return np.ascontiguousarray(W.astype(np.float32))


def _scaled_boxes(boxes, img_h, img_w, in_h, in_w):
    sx = np.float32(img_w / in_w)
    sy = np.float32(img_h / in_h)
    b = boxes.astype(np.float32) * np.array([sx, sy, sx, sy], np.float32)
    x0 = np.clip(b[:, 0], np.float32(0.0), np.float32(img_w))
    y0 = np.clip(b[:, 1], np.float32(0.0), np.float32(img_h))
    x1 = np.clip(b[:, 2], np.float32(0.0), np.float32(img_w))
    y1 = np.clip(b[:, 3], np.float32(0.0), np.float32(img_h))
    return x0, y0, x1, y1


def _prep_common(masks, boxes, img_h, img_w, in_h, in_w):
    x0, y0, x1, y1 = _scaled_boxes(boxes, img_h, img_w, in_h, in_w)
    xmat = _interp_mats(x0, x1, img_w, WM)   # [N, 28, img_w]
    ytmat = _interp_mats(y0, y1, img_h, HM)  # [N, 28, img_h]
    return xmat, ytmat


def _spans(mat):
    """Per-instance [start, span] of nonzero columns of mat [N, 28, S]."""
    n = mat.shape[0]
    s0 = np.zeros(n, np.int64)
    sl = np.zeros(n, np.int64)
    nz = mat.any(axis=1)
    for i in range(n):
        w = np.flatnonzero(nz[i])
        if w.size:
            s0[i], sl[i] = w[0], w[-1] - w[0] + 1
    return s0, sl


def _slot_geometry(rh_by_core):
    """rh_by_core [M, ni] row-spans -> per-core slot order + static slot
    row budgets RHT (multiple of TILE_H).  Slot pairs are arranged
    mid-size first, largest next, smallest last, so output DMAs start
    flowing early and the final transfer is small."""
    order = np.argsort(-rh_by_core, axis=1, kind="stable")   # [M, ni]
    ni = rh_by_core.shape[1]
    npair = ni // GSLOT
    if npair >= 4:
        pperm = [2, 0, 1] + list(range(3, npair))
    else:
        pperm = list(range(npair))
    sperm = np.concatenate(
        [np.arange(p * GSLOT, (p + 1) * GSLOT) for p in pperm]
    )
    order = order[:, sperm]
    sorted_rh = np.take_along_axis(rh_by_core, order, axis=1)
    rh_slot = sorted_rh.max(axis=0)                          # [ni]
    tiles = np.maximum((rh_slot + TILE_H - 1) // TILE_H, 1)
    rht = tiles * TILE_H
    return order, rht.astype(int), tiles.astype(int)


def _build_slotted(ni, img_h, img_w, rht, tiles):
    import concourse.bass as bass
    import concourse.mybir as mybir
    from concourse.tile import TileContext

    f32 = mybir.dt.float32
    bf16 = mybir.dt.bfloat16
    i32 = mybir.dt.int32
    nslot = len(rht)
    ngrp = nslot // GSLOT
    wg = [int(max(rht[g * GSLOT : (g + 1) * GSLOT])) for g in range(ngrp)]
    # my (=mblk|ybuf) column layout per group: [mask blockdiag KD | Y, KD x WG]
    myoff = []
    o = 0
    for g in range(ngrp):
        myoff.append(o)
        o += KD + wg[g]
    mytot = o
    segoff = np.concatenate([[0], np.cumsum(tiles)]).astype(int)
    nseg = int(segoff[-1])

    nc = bass.Bass()
    my_d = nc.dram_tensor("my", [KD, mytot], bf16, kind="ExternalInput")
    x_d = nc.dram_tensor("xb", [KD, ngrp * L], bf16, kind="ExternalInput")
    off_d = nc.dram_tensor("offs", [1, nslot], i32, kind="ExternalInput")
    out_d = nc.dram_tensor("out", [ni, img_h, img_w], f32, kind="ExternalOutput")

    with TileContext(nc) as tc:
        with (
            tc.tile_pool(name="my", bufs=1) as myp,
            tc.tile_pool(name="xb", bufs=1) as xbp,
            tc.tile_pool(name="of", bufs=1) as ofp,
            tc.tile_pool(name="mx", bufs=1) as mxp,
            tc.tile_pool(name="pay", bufs=1) as payp,
            tc.tile_pool(name="psA", bufs=2, space="PSUM") as psa,
            tc.tile_pool(name="psB", bufs=3, space="PSUM") as psb,
        ):
            myt = myp.tile([KD, mytot], bf16, tag="my")
            xbt = xbp.tile([KD, ngrp * L], bf16, tag="xb")
            oft = ofp.tile([1, nslot], i32, tag="of")
            mxT = mxp.tile([KD, int(sum(wg))], bf16, tag="mxT")
            pay = payp.tile([TILE_H, nseg * L], f32, tag="pay")

            # my (stage-1 inputs) on sync, 2 groups per slice, in processing
            # order; xb + offs on scalar: the first matmul only waits for
            # its own my slice.
            half = (ngrp // 2) * L
            for h in range(0, ngrp, 2):
                a = myoff[h]
                b = myoff[h + 1] + KD + wg[h + 1]
                nc.sync.dma_start(out=myt[:, a:b], in_=my_d[:, a:b])
            nc.sync.dma_start(out=xbt[:, :half], in_=x_d[:, :half])
            nc.sync.dma_start(out=xbt[:, half:], in_=x_d[:, half:])
            nc.sync.dma_start(out=oft[:], in_=off_d[:])

            # Prefetch all destination offsets into engine registers up
            # front (value_load is ~0.7us) so out-DMA issue is just the
            # dma_start.  The register's engine must issue that slot's DMA.
            dengs = [(nc.sync, nc.gpsimd, nc.scalar, nc.gpsimd)[k % 4] for k in range(nslot)]
            rvs = [dengs[k].value_load(oft[0:1, k : k + 1]) for k in range(nslot)]

            import bass_rust

            cpi = 0
            mxo = [0]
            for g in range(1, ngrp):
                mxo.append(mxo[-1] + wg[g - 1])
            for g in range(ngrp):
                a = myoff[g]
                pa = psa.tile([KD, 512], f32, tag="pa")
                nc.tensor.matmul(
                    out=pa[:, : wg[g]],
                    lhsT=myt[:, a : a + KD],
                    rhs=myt[:, a + KD : a + KD + wg[g]],
                    start=True,
                    stop=True,
                )
                enga = nc.vector.tensor_copy
                enga(out=mxT[:, mxo[g] : mxo[g] + wg[g]], in_=pa[:, : wg[g]])
                for i in range(GSLOT):
                    k = g * GSLOT + i
                    nt = int(tiles[k])
                    s0 = int(segoff[k])
                    # pairs of row-tiles share one 2-bank PSUM tile and one
                    # strided copy into the payload
                    for t0 in range(0, nt, 2):
                        npair = min(2, nt - t0)
                        pb = psb.tile([TILE_H, 1024], f32, tag="pb")
                        for h in range(npair):
                            t = t0 + h
                            nc.tensor.matmul(
                                out=pb[:, h * 512 : h * 512 + L],
                                lhsT=mxT[
                                    i * PB : i * PB + HM,
                                    mxo[g] + t * TILE_H : mxo[g] + (t + 1) * TILE_H,
                                ],
                                rhs=xbt[i * PB : i * PB + HM, g * L : (g + 1) * L],
                                start=True,
                                stop=True,
                            )
                        s = s0 + t0
                        engb = (nc.vector.tensor_copy, nc.scalar.copy)[cpi % 2]
                        cpi += 1
                        if npair == 2:
                            src_ps = pb[:, 0:L].copy()
                            src_ps.ap = bass_rust.VecI64Pair(
                                [[1024, TILE_H], [512, 2], [1, L]]
                            )
                            dst = pay[:, s * L : (s + 2) * L].rearrange(
                                "p (b c) -> p b c", b=2
                            )
                            engb(out=dst, in_=src_ps)
                        else:
                            engb(out=pay[:, s * L : (s + 1) * L], in_=pb[:, :L])
                    # one windowed write per slot: [RHT_k, L] at runtime offset
                    tmpl = out_d[0, 0 : nt * TILE_H, 0:L].rearrange(
                        "(t p) l -> p t l", p=TILE_H
                    )
                    dyn = bass.AP(
                        tensor=tmpl.tensor,
                        offset=rvs[k],
                        ap=tmpl.ap,
                        dep_tracking_offset=k * 3 * TILE_H * img_w,
                    )
                    src = pay[:, s0 * L : (s0 + nt) * L].rearrange(
                        "p (t l) -> p t l", l=L
                    )
                    dengs[k].dma_start(out=dyn, in_=src)
    _split_multi_waits(nc)
    return nc


def _build_dense(ni, img_h, img_w):
    """Fallback: writes every output pixel (no window assumption)."""
    import concourse.bass as bass
    import concourse.mybir as mybir
    from concourse.tile import TileContext

    f32 = mybir.dt.float32
    f32r = mybir.dt.float32r
    nc = bass.Bass()
    maskT_d = nc.dram_tensor("maskT", [ni, WM, HM], f32r, kind="ExternalInput")
    x_d = nc.dram_tensor("xmat", [ni, WM, img_w], f32r, kind="ExternalInput")
    yt_d = nc.dram_tensor("ytmat", [ni, HM, img_h], f32r, kind="ExternalInput")
    out_d = nc.dram_tensor("out", [ni, img_h, img_w], f32, kind="ExternalOutput")
    chunks = []
    c = 0
    while c < img_w:
        cw = min(512, img_w - c)
        chunks.append((c, cw))
        c += cw
    rtiles = []
    r = 0
    while r < img_h:
        rh = min(128, img_h - r)
        rtiles.append((r, rh))
        r += rh

    with TileContext(nc) as tc:
        with (
            tc.tile_pool(name="w", bufs=3) as wp,
            tc.tile_pool(name="mx", bufs=2) as mxp,
            tc.tile_pool(name="psA", bufs=2, space="PSUM") as psa,
            tc.tile_pool(name="psB", bufs=2, space="PSUM") as psb,
            tc.tile_pool(name="ob", bufs=4) as obp,
        ):
            for n in range(ni):
                mT = wp.tile([WM, HM], f32r, tag="mT")
                xt = wp.tile([WM, img_w], f32r, tag="xt")
                yt = wp.tile([HM, img_h], f32r, tag="yt")
                nc.sync.dma_start(out=mT[:], in_=maskT_d[n])
                nc.sync.dma_start(out=xt[:], in_=x_d[n])
                nc.sync.dma_start(out=yt[:], in_=yt_d[n])

                mx = mxp.tile([HM, img_w], f32r, tag="mx")
                for j, (c0, cw) in enumerate(chunks):
                    pa = psa.tile([HM, 512], f32, tag="pa")
                    nc.tensor.matmul(
                        out=pa[:, :cw], lhsT=mT[:], rhs=xt[:, c0 : c0 + cw],
                        start=True, stop=True,
                    )
                    if j % 2 == 0:
                        nc.vector.tensor_copy(out=mx[:, c0 : c0 + cw], in_=pa[:, :cw])
                    else:
                        nc.scalar.copy(out=mx[:, c0 : c0 + cw], in_=pa[:, :cw])

                for r0, rh in rtiles:
                    pb = psb.tile([128, 3 * 512], f32, tag="pb")
                    for k, (c0, cw) in enumerate(chunks):
                        nc.tensor.matmul(
                            out=pb[:rh, k * 512 : k * 512 + cw],
                            lhsT=yt[:, r0 : r0 + rh],
                            rhs=mx[:, c0 : c0 + cw],
                            start=True, stop=True,
                        )
                    ob = obp.tile([128, img_w], f32, tag="ob")
                    for k, (c0, cw) in enumerate(chunks):
                        eng = nc.vector.tensor_copy if k % 2 == 0 else nc.scalar.copy
                        eng(out=ob[:rh, c0 : c0 + cw], in_=pb[:rh, k * 512 : k * 512 + cw])
                    nc.sync.dma_start(out=out_d[n, r0 : r0 + rh, :], in_=ob[:rh, :])
    _split_multi_waits(nc)
    return nc


def _run(masks, boxes, img_h, img_w, in_h, in_w, trace=False):
    n = masks.shape[0]
    assert n % N_CORES == 0
    ni = n // N_CORES
    xmat, ytmat = _prep_common(masks, boxes, img_h, img_w, in_h, in_w)
    c0s, cws = _spans(xmat)
    r0s, rhs_ = _spans(ytmat)

    fits = (
        ni % GSLOT == 0
        and img_w >= L
        and cws.max(initial=0) <= L
        and rhs_.max(initial=0) <= 3 * TILE_H
        and img_h >= 3 * TILE_H
    )
    if fits:
        rh_by_core = rhs_.reshape(N_CORES, ni)
        order, rht, tiles = _slot_geometry(rh_by_core)
        if rht.max() <= img_h:
            return _run_slotted(
                masks, xmat, ytmat, c0s, r0s, rhs_, order, rht, tiles,
                ni, img_h, img_w, trace,
            )
    return _run_dense(masks, xmat, ytmat, ni, img_h, img_w, trace)


def _run_slotted(masks, xmat, ytmat, c0s, r0s, rhs_, order, rht, tiles,
                 ni, img_h, img_w, trace):
    import ml_dtypes
    from concourse.bass_utils import run_bass_kernel_spmd

    bf16 = ml_dtypes.bfloat16
    key = ("slot", ni, img_h, img_w, tuple(rht))
    if key not in _BUILD_CACHE:
        _BUILD_CACHE[key] = _build_slotted(ni, img_h, img_w, rht, tiles)
    nc = _BUILD_CACHE[key]

    nslot = ni
    ngrp = nslot // GSLOT
    wg = [int(max(rht[g * GSLOT : (g + 1) * GSLOT])) for g in range(ngrp)]
    myoff = []
    o = 0
    for g in range(ngrp):
        myoff.append(o)
        o += KD + wg[g]
    mytot = o

    in_maps = []
    for c in range(N_CORES):
        my = np.zeros((KD, mytot), bf16)
        xb = np.zeros((KD, ngrp * L), bf16)
        offs = np.zeros((1, nslot), np.int32)
        for k in range(nslot):
            g, i = divmod(k, GSLOT)
            n = c * ni + int(order[c, k])
            nloc = int(order[c, k])
            a = myoff[g]
            p0 = i * PB
            my[p0 : p0 + HM, a + p0 : a + p0 + HM] = masks[n, 0].astype(bf16)
            rr = min(int(r0s[n]), img_h - int(rht[k]))
            take = min(wg[g], img_h - rr)
            my[p0 : p0 + HM, a + KD : a + KD + take] = ytmat[n][
                :, rr : rr + take
            ].astype(bf16)
            cc = min(int(c0s[n]), img_w - L)
            xb[p0 : p0 + HM, g * L : (g + 1) * L] = xmat[n][:, cc : cc + L].astype(
                bf16
            )
            offs[0, k] = nloc * img_h * img_w + rr * img_w + cc
        in_maps.append({"my": my, "xb": xb, "offs": offs})

    res = run_bass_kernel_spmd(nc, in_maps, core_ids=list(range(N_CORES)), trace=trace)
    out = np.concatenate([res.results[c]["out"] for c in range(N_CORES)], axis=0)
    return out, res


def _run_dense(masks, xmat, ytmat, ni, img_h, img_w, trace):
    from concourse.bass_utils import run_bass_kernel_spmd

    key = ("dense", ni, img_h, img_w)
    if key not in _BUILD_CACHE:
        _BUILD_CACHE[key] = _build_dense(ni, img_h, img_w)
    nc = _BUILD_CACHE[key]
    maskt = np.ascontiguousarray(
        np.transpose(masks[:, 0].astype(np.float32), (0, 2, 1))
    )
    in_maps = []
    for c in range(N_CORES):
        s = slice(c * ni, (c + 1) * ni)
        in_maps.append({"maskT": maskt[s], "xmat": xmat[s], "ytmat": ytmat[s]})
    res = run_bass_kernel_spmd(nc, in_maps, core_ids=list(range(N_CORES)), trace=trace)
    out = np.concatenate([res.results[c]["out"] for c in range(N_CORES)], axis=0)
    return out, res


def kernel(masks, boxes, img_h, img_w, in_h, in_w):
    img_h, img_w, in_h, in_w = int(img_h), int(img_w), int(in_h), int(in_w)
    masks = np.asarray(masks, dtype=np.float32)
    boxes = np.asarray(boxes, dtype=np.float32)
    out, _ = _run(masks, boxes, img_h, img_w, in_h, in_w, trace=False)
    return out
